# revision 48
# baseline (speedup 1.0000x reference)
"""Causal self-attention (B=2,S=2048,C=768,H=12,D=64) on 8 trn2 NeuronCores.

Sharding: core c -> batch b=c//4, head-group g=c%4 (3 heads each).
Per-core program (SPMD, same NEFF, different data):
  A: qk^T [feat, s] = Wqk^T x^T via fp8e4 DoubleRow matmuls (2x PE rate),
     evicted to fp8 with bias add, alternating ACT/DVE.
  R: SBUF->SBUF DMAs relayout q,k into [32, 2, S] DoubleRow operand form.
  B: V [t, hd] = x W_v (bf16) + ones col -> softmax denominators for free.
  C: per (s-block j, head): scores^T tiles [t,s] via fp8 DoubleRow, exp on
     ACT (pair-merged [128,1024] activations; OFFLOAD can reroute pairs to a
     DVE Schraudolph int-trick exp + Pool convert, currently disabled as the
     added latency outweighed the ACT relief); causal via suffix-trim +
     gpsimd triangular select; PV: out^T[d,s] + denom row accumulated over
     t on PE; normalize r=1/denom (DVE), gpsimd partition_broadcast,
     out_norm = out^T * r (DVE).
  D: proj-T summed over heads: y^T[c,s] = sum_h Wp_h^T outn_h^T (bf16).
Host: y[b] = sum_g y_g^T.T + b_proj.
"""

import numpy as np
import ml_dtypes

S = 2048
C = 768
D = 64
HL = 3
KC = C // 128
KP = C // 256
NB = S // 512
TT = S // 128
SB = 512

# Schraudolph exp(x/8) ~= bitcast_f32(int32(x*SCH_A + SCH_B))
SCH_A = 12102203.161561485 * 0.125
SCH_B = 1064986316.0

# exp offload to DVE(ts)+Pool(convert): tail blocks where ACT is the choke.
OFFLOAD = set()


def _slot(f):
    # feature slot f (0..5 = q0 k0 q1 k1 q2 k2) -> (nf tile, offset)
    # nf group h = head h's q (offset 0) + k (offset 64), so each head's
    # scores unblock after a single phase-A group
    return f // 2, 64 * (f % 2)


def _qkslot(f):
    # qkpk destination: q_h -> slot h, k_h -> slot 3+h
    return (f // 2) if f % 2 == 0 else (3 + f // 2)


_NC_CACHE = {}


def _build_nc():
    import concourse.bass as bass
    import concourse.tile as tile
    import concourse.mybir as mybir
    from concourse import bacc

    bf16 = mybir.dt.bfloat16
    fp8 = mybir.dt.float8e4
    f32 = mybir.dt.float32
    i32 = mybir.dt.int32
    AF = mybir.ActivationFunctionType
    DR = mybir.MatmulPerfMode.DoubleRow
    ALU = mybir.AluOpType

    nc = bacc.Bacc(
        "TRN2",
        target_bir_lowering=False,
        debug=False,
        enable_asserts=False,
        num_devices=8,
    )
    xpk = nc.declare_dram_parameter("xpk", [128, KP, 2, S], fp8, isOutput=False)
    xT = nc.declare_dram_parameter("xT", [C, S], bf16, isOutput=False)
    wqkpk = nc.declare_dram_parameter("wqkpk", [128, KP, 2, 6 * D], fp8, isOutput=False)
    wv = nc.declare_dram_parameter("wv", [C, HL * D], bf16, isOutput=False)
    wp01 = nc.declare_dram_parameter("wp01", [128, C], bf16, isOutput=False)
    wp2 = nc.declare_dram_parameter("wp2", [D, C], bf16, isOutput=False)
    bqk = nc.declare_dram_parameter("bqk", [6 * D], f32, isOutput=False)
    bv = nc.declare_dram_parameter("bv", [HL * D], f32, isOutput=False)
    yT = nc.declare_dram_parameter("yT", [C, S], bf16, isOutput=True)

    with tile.TileContext(nc) as tc:
        with (
            tc.tile_pool(name="const", bufs=1) as cpool,
            tc.tile_pool(name="exp", bufs=5) as epool,
            tc.tile_pool(name="exf", bufs=2) as xfpool,
            tc.tile_pool(name="ou", bufs=4) as oupool,
            tc.tile_pool(name="rr", bufs=4) as rpool,
            tc.tile_pool(name="ysb", bufs=6) as ypool,
            tc.tile_pool(name="ps1", bufs=2, space="PSUM") as ps1,
            tc.tile_pool(name="psC", bufs=2, space="PSUM") as psC,
            tc.tile_pool(name="psPV", bufs=2, space="PSUM") as psPV,
        ):
            # ---- persistent SBUF; qk-path inputs first on SP queue ----
            wqk_sb = cpool.tile([128, KP, 2, 6 * D], fp8)
            nc.sync.dma_start(wqk_sb, wqkpk[:])
            bqk_sb = cpool.tile([128, 3], f32)
            nc.sync.dma_start(bqk_sb, bqk.rearrange("(o p) -> p o", p=128))
            xpk_sb = cpool.tile([128, KP, 2, S], fp8)
            for j in range(NB):
                nc.sync.dma_start(
                    xpk_sb[:, :, :, j * SB : (j + 1) * SB],
                    xpk[:, :, :, j * SB : (j + 1) * SB],
                )
            xt_sb = cpool.tile([128, KC, S], bf16)
            xTr = xT.rearrange("(o p) s -> p o s", p=128)
            wv_sb = cpool.tile([128, KC, HL * D], bf16)
            wp01_sb = cpool.tile([128, C], bf16)
            wp2_sb = cpool.tile([D, C], bf16)
            bv_sb = cpool.tile([128, HL * D], f32)

            def late_loads():
                # SP queue, emitted after the relayouts: SP head-of-line
                # blocking on the relayout waits delays these transfers so
                # the latency-critical relayouts reach the DMA engines first.
                for j in range(NB):
                    nc.sync.dma_start(
                        xt_sb[:, :, j * SB : (j + 1) * SB],
                        xTr[:, :, j * SB : (j + 1) * SB],
                    )
                nc.sync.dma_start(wv_sb, wv.rearrange("(o p) n -> p o n", p=128))
                nc.sync.dma_start(bv_sb, bv[None, :].to_broadcast((128, HL * D)))
                nc.sync.dma_start(wp01_sb, wp01[:])
                nc.sync.dma_start(wp2_sb, wp2[:])

            qk_sb = cpool.tile([128, 3, S], fp8)
            qkpk = cpool.tile([32, 6, 2, S], fp8)
            v1_sb = cpool.tile([128, TT, HL, D + 1], bf16)
            nc.gpsimd.memset(v1_sb[:, :, :, D : D + 1], 1.0)  # ones col only

            # ---- phase A: qk^T = Wqk^T @ x^T (fp8 DoubleRow) ----
            def phase_a(nf):
                for j in range(NB):
                    ps = ps1.tile([128, SB], f32, tag="p1", name=f"psa{nf}_{j}")
                    for kp in range(KP):
                        nc.tensor.matmul(
                            ps,
                            wqk_sb[:, kp, :, nf * 128 : (nf + 1) * 128],
                            xpk_sb[:, kp, :, j * SB : (j + 1) * SB],
                            start=(kp == 0),
                            stop=(kp == KP - 1),
                            perf_mode=DR,
                        )
                    dst = qk_sb[:, nf, j * SB : (j + 1) * SB]
                    if j % 2 == 0 and nf < 2:
                        # ACT evicts only while ACT is still idle; nf2's
                        # window collides with the first exps
                        nc.scalar.activation(
                            dst, ps, AF.Identity, bias=bqk_sb[:, nf : nf + 1]
                        )
                    else:
                        nc.vector.tensor_scalar(
                            dst, ps, bqk_sb[:, nf : nf + 1], None, op0=ALU.add
                        )

            def relayout(f, j0=0, j1=NB):
                nf, o = _slot(f)
                d = _qkslot(f)
                for i in range(2):
                    nc.sync.dma_start(
                        qkpk[:, d, i, j0 * SB : j1 * SB],
                        qk_sb[o + 32 * i : o + 32 * (i + 1), nf, j0 * SB : j1 * SB],
                    )

            # ---- phase B: V = x W_v (bf16) ----
            def phase_b(tt0, tt1):
                for tt in range(tt0, tt1):
                    ps = ps1.tile([128, SB], f32, tag="p1", name=f"psb{tt}")
                    for kc in range(KC):
                        nc.tensor.matmul(
                            ps[:, 0 : HL * D],
                            xt_sb[:, kc, tt * 128 : (tt + 1) * 128],
                            wv_sb[:, kc, :],
                            start=(kc == 0),
                            stop=(kc == KC - 1),
                        )
                    nc.vector.tensor_add(
                        v1_sb[:, tt, :, 0:D],
                        ps[:, 0 : HL * D].rearrange("p (h d) -> p h d", h=HL),
                        bv_sb.rearrange("p (h d) -> p h d", h=HL),
                    )

            # ---- scores^T pair (2 t-tiles) -> exp ----
            def scores_pair(j, h, p, ex):
                sc = psC.tile([128, 2 * SB], f32, tag="c", name=f"c{j}_{h}_{p}")
                los = []
                for k in range(2):
                    tt = 2 * p + k
                    i = tt - 4 * j
                    lo = 128 * i if i > 0 else 0
                    los.append(lo)
                    nc.tensor.matmul(
                        sc[:, k * SB + lo : (k + 1) * SB],
                        qkpk[:, 3 + h, :, tt * 128 : (tt + 1) * 128],
                        qkpk[:, h, :, j * SB + lo : (j + 1) * SB],
                        start=True,
                        stop=True,
                        perf_mode=DR,
                    )
                tt0 = 2 * p
                if (j, h, p) in OFFLOAD and los[1] == 0 and los[0] == 0:
                    ef = xfpool.tile([128, 2 * SB], f32, tag="exf", name=f"ef{j}{h}{p}")
                    nc.vector.tensor_scalar(
                        ef.bitcast(i32), sc, SCH_A, SCH_B, op0=ALU.mult, op1=ALU.add
                    )
                    nc.gpsimd.tensor_copy(ex[:, tt0 * SB : tt0 * SB + 2 * SB], ef)
                elif los[1] == 0 or los[0] == 0:
                    # both tiles full, or diag pair A: one merged activation
                    # (tile1's cols < lo hold exp(stale-psum), never read by PV)
                    nc.scalar.activation(
                        ex[:, tt0 * SB : tt0 * SB + 2 * SB], sc, AF.Exp, scale=0.125
                    )
                else:
                    # diag pair B (trims 256,384): separate trimmed exps
                    for k in range(2):
                        tt = 2 * p + k
                        lo = los[k]
                        nc.scalar.activation(
                            ex[:, tt * SB + lo : (tt + 1) * SB],
                            sc[:, k * SB + lo : (k + 1) * SB],
                            AF.Exp,
                            scale=0.125,
                        )

                for k in range(2):
                    tt = 2 * p + k
                    i = tt - 4 * j
                    if i >= 0:
                        st = tt * SB + 128 * i
                        nc.gpsimd.affine_select(
                            out=ex[:, st : st + 128],
                            in_=ex[:, st : st + 128],
                            compare_op=ALU.is_ge,
                            fill=0.0,
                            base=0,
                            pattern=[[1, 128]],
                            channel_multiplier=-1,
                        )

            def pv_start(j, h):
                return psPV.tile([D + 1, SB], f32, tag="pv", name=f"po{j}_{h}")

            def pv_pair(j, h, p, po, ex, npair):
                for k in range(2):
                    tt = 2 * p + k
                    i = tt - 4 * j
                    lo = 128 * i if i > 0 else 0
                    nc.tensor.matmul(
                        po[:, lo:SB],
                        v1_sb[:, tt, h, :],
                        ex[:, tt * SB + lo : (tt + 1) * SB],
                        start=(tt == 0),
                        stop=(tt == 2 * npair - 1),
                    )

            def pv_norm(j, h, po, dst):
                r = rpool.tile([1, SB], f32, tag="r", name=f"r{j}_{h}")
                nc.vector.reciprocal(r, po[D : D + 1, :])
                rb = rpool.tile([D, SB], f32, tag="rb", name=f"rb{j}_{h}")
                nc.gpsimd.partition_broadcast(rb, r)
                nc.vector.tensor_mul(dst, po[0:D, :], rb)

            def pv_head(j, h, ex, dst):
                npair = 2 * (j + 1)
                po = pv_start(j, h)
                for p in range(npair):
                    pv_pair(j, h, p, po, ex, npair)
                pv_norm(j, h, po, dst)

            def proj_j(j, comb, oun2, evict_act=False):
                for ct in range(KC):
                    py = ps1.tile([128, SB], f32, tag="p1", name=f"py{j}_{ct}")
                    nc.tensor.matmul(
                        py,
                        wp01_sb[:, ct * 128 : (ct + 1) * 128],
                        comb,
                        start=True,
                        stop=False,
                    )
                    nc.tensor.matmul(
                        py,
                        wp2_sb[:, ct * 128 : (ct + 1) * 128],
                        oun2,
                        start=False,
                        stop=True,
                    )
                    ys = ypool.tile([128, SB], bf16, tag="ysb", name=f"ys{j}_{ct}")
                    if evict_act and ct % 2 == 0:
                        nc.scalar.activation(ys, py, AF.Copy)
                    else:
                        nc.vector.tensor_copy(ys, py)
                    nc.sync.dma_start(
                        yT[ct * 128 : (ct + 1) * 128, j * SB : (j + 1) * SB], ys
                    )

            # j=0 staged against phases A/B so ACT starts early
            ex_j0 = [
                epool.tile([128, TT * SB], bf16, tag="exp", name=f"exj0_{hh}")
                for hh in range(3)
            ]
            phase_a(0)  # q0 q1
            relayout(0)  # q0: only needs nf0 evicts -> start its DMA early
            phase_a(1)  # q2 k0
            relayout(3)  # k0
            phase_a(2)  # k1 k2
            relayout(1)
            relayout(4)
            for p in range(2):
                scores_pair(0, 0, p, ex_j0[0])
            relayout(2)
            relayout(5)
            late_loads()
            for p in range(2):
                scores_pair(0, 1, p, ex_j0[1])
            scores_pair(0, 2, 0, ex_j0[2])
            scores_pair(0, 2, 1, ex_j0[2])
            # first j1 lookahead pair ahead of phase B so ACT stays fed
            ex01 = [
                epool.tile([128, TT * SB], bf16, tag="exp", name=f"exp1_{hh}")
                for hh in range(2)
            ]
            scores_pair(1, 0, 0, ex01[0])
            scores_pair(1, 1, 0, ex01[1])
            phase_b(0, 4)  # pv j0 only needs v1 tiles 0..3
            comb0 = oupool.tile([128, SB], bf16, tag="ou01", name="comb0")
            tmp0 = oupool.tile([D, SB], bf16, tag="outmp", name="tmp0")
            pv_head(0, 0, ex_j0[0], comb0[0:D, :])
            phase_b(4, 6)
            pv_head(0, 1, ex_j0[1], tmp0)
            nc.sync.dma_start(comb0[D:128, :], tmp0)
            scores_pair(1, 0, 1, ex01[0])
            scores_pair(1, 1, 1, ex01[1])
            phase_b(6, 8)
            o2_0 = oupool.tile([D, SB], bf16, tag="ou2", name="o2_0")
            pv_head(0, 2, ex_j0[2], o2_0)
            scores_pair(1, 0, 2, ex01[0])
            scores_pair(1, 1, 2, ex01[1])
            phase_b(8, 10)
            scores_pair(1, 0, 3, ex01[0])
            scores_pair(1, 1, 3, ex01[1])
            phase_b(10, 12)
            # pre-emit the first h2 pairs of block 1 so they don't queue
            # behind proj_j(0)/phase_b in PE order at the block handoff
            NPRE = 2
            ex2_pre = epool.tile([128, TT * SB], bf16, tag="exp", name="ex2p_1")
            for p in range(NPRE):
                scores_pair(1, 2, p, ex2_pre)
            proj_j(0, comb0, o2_0)
            phase_b(12, TT)

            for j in range(1, NB):
                npair = 2 * (j + 1)
                comb = oupool.tile([128, SB], bf16, tag="ou01", name=f"comb{j}")
                tmp = oupool.tile([D, SB], bf16, tag="outmp", name=f"tmp{j}")
                pv_head(j, 0, ex01[0], comb[0:D, :])
                ex2 = ex2_pre
                if j == NB - 1:
                    # tail: get pv h1 (whose norm feeds comb -> proj) out
                    # ahead of the psC-throttled remaining h2 score stream
                    scores_pair(j, 2, NPRE, ex2)
                    scores_pair(j, 2, NPRE + 1, ex2)
                    pv_head(j, 1, ex01[1], tmp)
                    nc.sync.dma_start(comb[D:128, :], tmp)
                    for p in range(NPRE + 2, npair):
                        scores_pair(j, 2, p, ex2)
                else:
                    for p in range(NPRE, npair):
                        scores_pair(j, 2, p, ex2)
                    pv_head(j, 1, ex01[1], tmp)
                    nc.sync.dma_start(comb[D:128, :], tmp)
                    ex01 = [
                        epool.tile(
                            [128, TT * SB], bf16, tag="exp", name=f"exn{j}_{hh}"
                        )
                        for hh in range(2)
                    ]
                    for p in range(2 * (j + 2)):
                        scores_pair(j + 1, 0, p, ex01[0])
                        scores_pair(j + 1, 1, p, ex01[1])
                o2 = oupool.tile([D, SB], bf16, tag="ou2", name=f"o2_{j}")
                pv_head(j, 2, ex2, o2)
                if j < NB - 1:
                    # ex2_j's epool slot frees after pv_head(j,2): start the
                    # next block's h2 exps before proj to bridge the handoff
                    ex2_pre = epool.tile(
                        [128, TT * SB], bf16, tag="exp", name=f"ex2p_{j+1}"
                    )
                    for p in range(NPRE):
                        scores_pair(j + 1, 2, p, ex2_pre)
                proj_j(j, comb, o2, evict_act=(j == NB - 1))
    nc.finalize()
    return nc


def _get_nc():
    if "nc" not in _NC_CACHE:
        _NC_CACHE["nc"] = _build_nc()
    return _NC_CACHE["nc"]


def kernel(x, W_attn, b_attn, W_proj, b_proj):
    from concourse.bass_utils import run_bass_kernel_spmd

    x = np.asarray(x, np.float32)
    W_attn = np.asarray(W_attn, np.float32)
    b_attn = np.asarray(b_attn, np.float32)
    W_proj = np.asarray(W_proj, np.float32)
    b_proj = np.asarray(b_proj, np.float32)
    bf = ml_dtypes.bfloat16
    f8 = ml_dtypes.float8_e4m3

    nc = _get_nc()
    in_maps = []
    for c in range(8):
        b, g = c // 4, c % 4
        cs = slice(192 * g, 192 * (g + 1))
        Wq = W_attn[:, 0 * C : 1 * C][:, cs]
        Wk = W_attn[:, 1 * C : 2 * C][:, cs]
        Wv = W_attn[:, 2 * C : 3 * C][:, cs]
        Wqk = np.concatenate(
            [
                Wq[:, 0:64], Wk[:, 0:64],
                Wq[:, 64:128], Wk[:, 64:128],
                Wq[:, 128:192], Wk[:, 128:192],
            ],
            axis=1,
        )  # [768, 384], head-major q/k interleave
        wqkpk = np.ascontiguousarray(
            Wqk.reshape(KP, 2, 128, 6 * D).transpose(2, 0, 1, 3)
        ).astype(f8)
        xb = x[b]  # [S, C]
        xpk = np.ascontiguousarray(
            xb.T.reshape(KP, 2, 128, S).transpose(2, 0, 1, 3)
        ).astype(f8)
        bq = b_attn[0:C][cs]
        bk = b_attn[C : 2 * C][cs]
        in_maps.append(
            {
                "xpk": xpk,
                "xT": np.ascontiguousarray(xb.T).astype(bf),
                "wqkpk": wqkpk,
                "wv": np.ascontiguousarray(Wv).astype(bf),
                "wp01": np.ascontiguousarray(W_proj[cs, :][0:128]).astype(bf),
                "wp2": np.ascontiguousarray(W_proj[cs, :][128:192]).astype(bf),
                "bqk": np.ascontiguousarray(
                    np.concatenate(
                        [bq[0:64], bk[0:64], bq[64:128], bk[64:128],
                         bq[128:192], bk[128:192]]
                    )
                ).astype(np.float32),
                "bv": np.ascontiguousarray(b_attn[2 * C : 3 * C][cs]).astype(
                    np.float32
                ),
            }
        )

    res = run_bass_kernel_spmd(nc, in_maps, list(range(8)))
    _NC_CACHE["last_result"] = res

    out = np.zeros((2, S, C), np.float32)
    for c in range(8):
        b = c // 4
        yTc = np.asarray(res.results[c]["yT"], dtype=np.float32)  # [C, S]
        out[b] += yTc.T
    out += b_proj[None, None, :]
    return out


# revision 49
# speedup vs baseline: 1.0012x; 1.0012x over previous
"""Causal self-attention (B=2,S=2048,C=768,H=12,D=64) on 8 trn2 NeuronCores.

Sharding: core c -> batch b=c//4, head-group g=c%4 (3 heads each).
Per-core program (SPMD, same NEFF, different data):
  A: qk^T [feat, s] = Wqk^T x^T via fp8e4 DoubleRow matmuls (2x PE rate),
     evicted to fp8 with bias add, alternating ACT/DVE.
  R: SBUF->SBUF DMAs relayout q,k into [32, 2, S] DoubleRow operand form.
  B: V [t, hd] = x W_v (bf16) + ones col -> softmax denominators for free.
  C: per (s-block j, head): scores^T tiles [t,s] via fp8 DoubleRow, exp on
     ACT (pair-merged [128,1024] activations; OFFLOAD can reroute pairs to a
     DVE Schraudolph int-trick exp + Pool convert, currently disabled as the
     added latency outweighed the ACT relief); causal via suffix-trim +
     gpsimd triangular select; PV: out^T[d,s] + denom row accumulated over
     t on PE; normalize r=1/denom (DVE), gpsimd partition_broadcast,
     out_norm = out^T * r (DVE).
  D: proj-T summed over heads: y^T[c,s] = sum_h Wp_h^T outn_h^T (bf16).
Host: y[b] = sum_g y_g^T.T + b_proj.
"""

import numpy as np
import ml_dtypes

S = 2048
C = 768
D = 64
HL = 3
KC = C // 128
KP = C // 256
NB = S // 512
TT = S // 128
SB = 512

# Schraudolph exp(x/8) ~= bitcast_f32(int32(x*SCH_A + SCH_B))
SCH_A = 12102203.161561485 * 0.125
SCH_B = 1064986316.0

# exp offload to DVE(ts)+Pool(convert): tail blocks where ACT is the choke.
OFFLOAD = set()


def _slot(f):
    # feature slot f (0..5 = q0 k0 q1 k1 q2 k2) -> (nf tile, offset)
    # nf group h = head h's q (offset 0) + k (offset 64), so each head's
    # scores unblock after a single phase-A group
    return f // 2, 64 * (f % 2)


def _qkslot(f):
    # qkpk destination: q_h -> slot h, k_h -> slot 3+h
    return (f // 2) if f % 2 == 0 else (3 + f // 2)


_NC_CACHE = {}


def _build_nc():
    import concourse.bass as bass
    import concourse.tile as tile
    import concourse.mybir as mybir
    from concourse import bacc

    bf16 = mybir.dt.bfloat16
    fp8 = mybir.dt.float8e4
    f32 = mybir.dt.float32
    i32 = mybir.dt.int32
    AF = mybir.ActivationFunctionType
    DR = mybir.MatmulPerfMode.DoubleRow
    ALU = mybir.AluOpType

    nc = bacc.Bacc(
        "TRN2",
        target_bir_lowering=False,
        debug=False,
        enable_asserts=False,
        num_devices=8,
    )
    xpk = nc.declare_dram_parameter("xpk", [128, KP, 2, S], fp8, isOutput=False)
    xT = nc.declare_dram_parameter("xT", [C, S], bf16, isOutput=False)
    wqkpk = nc.declare_dram_parameter("wqkpk", [128, KP, 2, 6 * D], fp8, isOutput=False)
    wv = nc.declare_dram_parameter("wv", [C, HL * D], bf16, isOutput=False)
    wp01 = nc.declare_dram_parameter("wp01", [128, C], bf16, isOutput=False)
    wp2 = nc.declare_dram_parameter("wp2", [D, C], bf16, isOutput=False)
    bqk = nc.declare_dram_parameter("bqk", [6 * D], f32, isOutput=False)
    bv = nc.declare_dram_parameter("bv", [HL * D], f32, isOutput=False)
    yT = nc.declare_dram_parameter("yT", [C, S], bf16, isOutput=True)

    with tile.TileContext(nc) as tc:
        with (
            tc.tile_pool(name="const", bufs=1) as cpool,
            tc.tile_pool(name="exp", bufs=5) as epool,
            tc.tile_pool(name="exf", bufs=2) as xfpool,
            tc.tile_pool(name="ou", bufs=4) as oupool,
            tc.tile_pool(name="rr", bufs=4) as rpool,
            tc.tile_pool(name="ysb", bufs=6) as ypool,
            tc.tile_pool(name="ps1", bufs=2, space="PSUM") as ps1,
            tc.tile_pool(name="psC", bufs=2, space="PSUM") as psC,
            tc.tile_pool(name="psPV", bufs=2, space="PSUM") as psPV,
        ):
            # ---- persistent SBUF; qk-path inputs first on SP queue ----
            wqk_sb = cpool.tile([128, KP, 2, 6 * D], fp8)
            nc.sync.dma_start(wqk_sb, wqkpk[:])
            bqk_sb = cpool.tile([128, 3], f32)
            nc.sync.dma_start(bqk_sb, bqk.rearrange("(o p) -> p o", p=128))
            xpk_sb = cpool.tile([128, KP, 2, S], fp8)
            for j in range(NB):
                nc.sync.dma_start(
                    xpk_sb[:, :, :, j * SB : (j + 1) * SB],
                    xpk[:, :, :, j * SB : (j + 1) * SB],
                )
            xt_sb = cpool.tile([128, KC, S], bf16)
            xTr = xT.rearrange("(o p) s -> p o s", p=128)
            wv_sb = cpool.tile([128, KC, HL * D], bf16)
            wp01_sb = cpool.tile([128, C], bf16)
            wp2_sb = cpool.tile([D, C], bf16)
            bv_sb = cpool.tile([128, HL * D], f32)

            def late_loads():
                # SP queue, emitted after the relayouts: SP head-of-line
                # blocking on the relayout waits delays these transfers so
                # the latency-critical relayouts reach the DMA engines first.
                for j in range(NB):
                    nc.sync.dma_start(
                        xt_sb[:, :, j * SB : (j + 1) * SB],
                        xTr[:, :, j * SB : (j + 1) * SB],
                    )
                nc.sync.dma_start(wv_sb, wv.rearrange("(o p) n -> p o n", p=128))
                nc.sync.dma_start(bv_sb, bv[None, :].to_broadcast((128, HL * D)))
                nc.sync.dma_start(wp01_sb, wp01[:])
                nc.sync.dma_start(wp2_sb, wp2[:])

            qk_sb = cpool.tile([128, 3, S], fp8)
            qkpk = cpool.tile([32, 6, 2, S], fp8)
            v1_sb = cpool.tile([128, TT, HL, D + 1], bf16)
            nc.gpsimd.memset(v1_sb[:, :, :, D : D + 1], 1.0)  # ones col only

            # ---- phase A: qk^T = Wqk^T @ x^T (fp8 DoubleRow) ----
            def phase_a(nf):
                for j in range(NB):
                    ps = ps1.tile([128, SB], f32, tag="p1", name=f"psa{nf}_{j}")
                    for kp in range(KP):
                        nc.tensor.matmul(
                            ps,
                            wqk_sb[:, kp, :, nf * 128 : (nf + 1) * 128],
                            xpk_sb[:, kp, :, j * SB : (j + 1) * SB],
                            start=(kp == 0),
                            stop=(kp == KP - 1),
                            perf_mode=DR,
                        )
                    dst = qk_sb[:, nf, j * SB : (j + 1) * SB]
                    if j % 2 == 0:
                        nc.scalar.activation(
                            dst, ps, AF.Identity, bias=bqk_sb[:, nf : nf + 1]
                        )
                    else:
                        nc.vector.tensor_scalar(
                            dst, ps, bqk_sb[:, nf : nf + 1], None, op0=ALU.add
                        )

            def relayout(f, j0=0, j1=NB):
                nf, o = _slot(f)
                d = _qkslot(f)
                for i in range(2):
                    nc.sync.dma_start(
                        qkpk[:, d, i, j0 * SB : j1 * SB],
                        qk_sb[o + 32 * i : o + 32 * (i + 1), nf, j0 * SB : j1 * SB],
                    )

            # ---- phase B: V = x W_v (bf16) ----
            def phase_b(tt0, tt1):
                for tt in range(tt0, tt1):
                    ps = ps1.tile([128, SB], f32, tag="p1", name=f"psb{tt}")
                    for kc in range(KC):
                        nc.tensor.matmul(
                            ps[:, 0 : HL * D],
                            xt_sb[:, kc, tt * 128 : (tt + 1) * 128],
                            wv_sb[:, kc, :],
                            start=(kc == 0),
                            stop=(kc == KC - 1),
                        )
                    nc.vector.tensor_add(
                        v1_sb[:, tt, :, 0:D],
                        ps[:, 0 : HL * D].rearrange("p (h d) -> p h d", h=HL),
                        bv_sb.rearrange("p (h d) -> p h d", h=HL),
                    )

            # ---- scores^T pair (2 t-tiles) -> exp ----
            def scores_pair(j, h, p, ex):
                sc = psC.tile([128, 2 * SB], f32, tag="c", name=f"c{j}_{h}_{p}")
                los = []
                for k in range(2):
                    tt = 2 * p + k
                    i = tt - 4 * j
                    lo = 128 * i if i > 0 else 0
                    los.append(lo)
                    nc.tensor.matmul(
                        sc[:, k * SB + lo : (k + 1) * SB],
                        qkpk[:, 3 + h, :, tt * 128 : (tt + 1) * 128],
                        qkpk[:, h, :, j * SB + lo : (j + 1) * SB],
                        start=True,
                        stop=True,
                        perf_mode=DR,
                    )
                tt0 = 2 * p
                if (j, h, p) in OFFLOAD and los[1] == 0 and los[0] == 0:
                    ef = xfpool.tile([128, 2 * SB], f32, tag="exf", name=f"ef{j}{h}{p}")
                    nc.vector.tensor_scalar(
                        ef.bitcast(i32), sc, SCH_A, SCH_B, op0=ALU.mult, op1=ALU.add
                    )
                    nc.gpsimd.tensor_copy(ex[:, tt0 * SB : tt0 * SB + 2 * SB], ef)
                elif los[1] == 0 or los[0] == 0:
                    # both tiles full, or diag pair A: one merged activation
                    # (tile1's cols < lo hold exp(stale-psum), never read by PV)
                    nc.scalar.activation(
                        ex[:, tt0 * SB : tt0 * SB + 2 * SB], sc, AF.Exp, scale=0.125
                    )
                else:
                    # diag pair B (trims 256,384): separate trimmed exps
                    for k in range(2):
                        tt = 2 * p + k
                        lo = los[k]
                        nc.scalar.activation(
                            ex[:, tt * SB + lo : (tt + 1) * SB],
                            sc[:, k * SB + lo : (k + 1) * SB],
                            AF.Exp,
                            scale=0.125,
                        )

                for k in range(2):
                    tt = 2 * p + k
                    i = tt - 4 * j
                    if i >= 0:
                        st = tt * SB + 128 * i
                        nc.gpsimd.affine_select(
                            out=ex[:, st : st + 128],
                            in_=ex[:, st : st + 128],
                            compare_op=ALU.is_ge,
                            fill=0.0,
                            base=0,
                            pattern=[[1, 128]],
                            channel_multiplier=-1,
                        )

            def pv_start(j, h):
                return psPV.tile([D + 1, SB], f32, tag="pv", name=f"po{j}_{h}")

            def pv_pair(j, h, p, po, ex, npair):
                for k in range(2):
                    tt = 2 * p + k
                    i = tt - 4 * j
                    lo = 128 * i if i > 0 else 0
                    nc.tensor.matmul(
                        po[:, lo:SB],
                        v1_sb[:, tt, h, :],
                        ex[:, tt * SB + lo : (tt + 1) * SB],
                        start=(tt == 0),
                        stop=(tt == 2 * npair - 1),
                    )

            def pv_norm(j, h, po, dst):
                r = rpool.tile([1, SB], f32, tag="r", name=f"r{j}_{h}")
                nc.vector.reciprocal(r, po[D : D + 1, :])
                rb = rpool.tile([D, SB], f32, tag="rb", name=f"rb{j}_{h}")
                nc.gpsimd.partition_broadcast(rb, r)
                nc.vector.tensor_mul(dst, po[0:D, :], rb)

            def pv_head(j, h, ex, dst):
                npair = 2 * (j + 1)
                po = pv_start(j, h)
                for p in range(npair):
                    pv_pair(j, h, p, po, ex, npair)
                pv_norm(j, h, po, dst)

            def proj_j(j, comb, oun2, evict_act=False):
                for ct in range(KC):
                    py = ps1.tile([128, SB], f32, tag="p1", name=f"py{j}_{ct}")
                    nc.tensor.matmul(
                        py,
                        wp01_sb[:, ct * 128 : (ct + 1) * 128],
                        comb,
                        start=True,
                        stop=False,
                    )
                    nc.tensor.matmul(
                        py,
                        wp2_sb[:, ct * 128 : (ct + 1) * 128],
                        oun2,
                        start=False,
                        stop=True,
                    )
                    ys = ypool.tile([128, SB], bf16, tag="ysb", name=f"ys{j}_{ct}")
                    if evict_act and ct % 2 == 0:
                        nc.scalar.activation(ys, py, AF.Copy)
                    else:
                        nc.vector.tensor_copy(ys, py)
                    nc.sync.dma_start(
                        yT[ct * 128 : (ct + 1) * 128, j * SB : (j + 1) * SB], ys
                    )

            # j=0 staged against phases A/B so ACT starts early
            ex_j0 = [
                epool.tile([128, TT * SB], bf16, tag="exp", name=f"exj0_{hh}")
                for hh in range(3)
            ]
            phase_a(0)  # q0 q1
            relayout(0)  # q0: only needs nf0 evicts -> start its DMA early
            phase_a(1)  # q2 k0
            relayout(3)  # k0
            phase_a(2)  # k1 k2
            relayout(1)
            relayout(4)
            for p in range(2):
                scores_pair(0, 0, p, ex_j0[0])
            relayout(2)
            relayout(5)
            late_loads()
            for p in range(2):
                scores_pair(0, 1, p, ex_j0[1])
            scores_pair(0, 2, 0, ex_j0[2])
            scores_pair(0, 2, 1, ex_j0[2])
            # first j1 lookahead pair ahead of phase B so ACT stays fed
            ex01 = [
                epool.tile([128, TT * SB], bf16, tag="exp", name=f"exp1_{hh}")
                for hh in range(2)
            ]
            scores_pair(1, 0, 0, ex01[0])
            scores_pair(1, 1, 0, ex01[1])
            phase_b(0, 4)  # pv j0 only needs v1 tiles 0..3
            comb0 = oupool.tile([128, SB], bf16, tag="ou01", name="comb0")
            tmp0 = oupool.tile([D, SB], bf16, tag="outmp", name="tmp0")
            pv_head(0, 0, ex_j0[0], comb0[0:D, :])
            phase_b(4, 6)
            pv_head(0, 1, ex_j0[1], tmp0)
            nc.sync.dma_start(comb0[D:128, :], tmp0)
            scores_pair(1, 0, 1, ex01[0])
            scores_pair(1, 1, 1, ex01[1])
            phase_b(6, 8)
            o2_0 = oupool.tile([D, SB], bf16, tag="ou2", name="o2_0")
            pv_head(0, 2, ex_j0[2], o2_0)
            scores_pair(1, 0, 2, ex01[0])
            scores_pair(1, 1, 2, ex01[1])
            phase_b(8, 10)
            scores_pair(1, 0, 3, ex01[0])
            scores_pair(1, 1, 3, ex01[1])
            phase_b(10, 12)
            # pre-emit the first h2 pairs of block 1 so they don't queue
            # behind proj_j(0)/phase_b in PE order at the block handoff
            NPRE = 2
            ex2_pre = epool.tile([128, TT * SB], bf16, tag="exp", name="ex2p_1")
            for p in range(NPRE):
                scores_pair(1, 2, p, ex2_pre)
            proj_j(0, comb0, o2_0)
            phase_b(12, TT)

            for j in range(1, NB):
                npair = 2 * (j + 1)
                comb = oupool.tile([128, SB], bf16, tag="ou01", name=f"comb{j}")
                tmp = oupool.tile([D, SB], bf16, tag="outmp", name=f"tmp{j}")
                pv_head(j, 0, ex01[0], comb[0:D, :])
                ex2 = ex2_pre
                if j == NB - 1:
                    # tail: get pv h1 (whose norm feeds comb -> proj) out
                    # ahead of the psC-throttled remaining h2 score stream
                    scores_pair(j, 2, NPRE, ex2)
                    scores_pair(j, 2, NPRE + 1, ex2)
                    pv_head(j, 1, ex01[1], tmp)
                    nc.sync.dma_start(comb[D:128, :], tmp)
                    for p in range(NPRE + 2, npair):
                        scores_pair(j, 2, p, ex2)
                else:
                    for p in range(NPRE, npair):
                        scores_pair(j, 2, p, ex2)
                    pv_head(j, 1, ex01[1], tmp)
                    nc.sync.dma_start(comb[D:128, :], tmp)
                    ex01 = [
                        epool.tile(
                            [128, TT * SB], bf16, tag="exp", name=f"exn{j}_{hh}"
                        )
                        for hh in range(2)
                    ]
                    for p in range(2 * (j + 2)):
                        scores_pair(j + 1, 0, p, ex01[0])
                        scores_pair(j + 1, 1, p, ex01[1])
                o2 = oupool.tile([D, SB], bf16, tag="ou2", name=f"o2_{j}")
                pv_head(j, 2, ex2, o2)
                if j < NB - 1:
                    # ex2_j's epool slot frees after pv_head(j,2): start the
                    # next block's h2 exps before proj to bridge the handoff
                    ex2_pre = epool.tile(
                        [128, TT * SB], bf16, tag="exp", name=f"ex2p_{j+1}"
                    )
                    for p in range(NPRE):
                        scores_pair(j + 1, 2, p, ex2_pre)
                proj_j(j, comb, o2, evict_act=(j == NB - 1))
    nc.finalize()
    return nc


def _get_nc():
    if "nc" not in _NC_CACHE:
        _NC_CACHE["nc"] = _build_nc()
    return _NC_CACHE["nc"]


def kernel(x, W_attn, b_attn, W_proj, b_proj):
    from concourse.bass_utils import run_bass_kernel_spmd

    x = np.asarray(x, np.float32)
    W_attn = np.asarray(W_attn, np.float32)
    b_attn = np.asarray(b_attn, np.float32)
    W_proj = np.asarray(W_proj, np.float32)
    b_proj = np.asarray(b_proj, np.float32)
    bf = ml_dtypes.bfloat16
    f8 = ml_dtypes.float8_e4m3

    nc = _get_nc()
    in_maps = []
    for c in range(8):
        b, g = c // 4, c % 4
        cs = slice(192 * g, 192 * (g + 1))
        Wq = W_attn[:, 0 * C : 1 * C][:, cs]
        Wk = W_attn[:, 1 * C : 2 * C][:, cs]
        Wv = W_attn[:, 2 * C : 3 * C][:, cs]
        Wqk = np.concatenate(
            [
                Wq[:, 0:64], Wk[:, 0:64],
                Wq[:, 64:128], Wk[:, 64:128],
                Wq[:, 128:192], Wk[:, 128:192],
            ],
            axis=1,
        )  # [768, 384], head-major q/k interleave
        wqkpk = np.ascontiguousarray(
            Wqk.reshape(KP, 2, 128, 6 * D).transpose(2, 0, 1, 3)
        ).astype(f8)
        xb = x[b]  # [S, C]
        xpk = np.ascontiguousarray(
            xb.T.reshape(KP, 2, 128, S).transpose(2, 0, 1, 3)
        ).astype(f8)
        bq = b_attn[0:C][cs]
        bk = b_attn[C : 2 * C][cs]
        in_maps.append(
            {
                "xpk": xpk,
                "xT": np.ascontiguousarray(xb.T).astype(bf),
                "wqkpk": wqkpk,
                "wv": np.ascontiguousarray(Wv).astype(bf),
                "wp01": np.ascontiguousarray(W_proj[cs, :][0:128]).astype(bf),
                "wp2": np.ascontiguousarray(W_proj[cs, :][128:192]).astype(bf),
                "bqk": np.ascontiguousarray(
                    np.concatenate(
                        [bq[0:64], bk[0:64], bq[64:128], bk[64:128],
                         bq[128:192], bk[128:192]]
                    )
                ).astype(np.float32),
                "bv": np.ascontiguousarray(b_attn[2 * C : 3 * C][cs]).astype(
                    np.float32
                ),
            }
        )

    res = run_bass_kernel_spmd(nc, in_maps, list(range(8)))
    _NC_CACHE["last_result"] = res

    out = np.zeros((2, S, C), np.float32)
    for c in range(8):
        b = c // 4
        yTc = np.asarray(res.results[c]["yT"], dtype=np.float32)  # [C, S]
        out[b] += yTc.T
    out += b_proj[None, None, :]
    return out
